# revision 30
# baseline (speedup 1.0000x reference)
"""BiLSTM (2-layer, H=50, D=207, T=30, B=16384) -> FC(2) Trainium2 kernel.

Data-parallel over 8 NeuronCores (2048 batch rows each). Host pre-packs
x into feature-major bf16 layout [T, 208, B] (ones row folds biases into
the input projection) and reorders weights into PSUM-bank gate layout.

Device layout (per core, 4 sweeps of 512 batch):
  - quadrant packing: partitions 0:50 = dir-f (or sweep s1), 64:114 =
    dir-b (or sweep s2); weight columns zero-padded so junk rows are 0.
  - Phase A: layer-0 fwd+bwd scans in lockstep (fwd eats x[t=k], bwd
    x[t=29-k]); gates accumulate in PSUM [128, 4banks]; ONE sigmoid
    ACTIVATE over all 4 banks -- the g-gate weight columns are
    pre-scaled x2 on host so tanh(ag) = 2*sigmoid(2ag)-1, recovered
    with a single 4x-mode tensor_scalar on DVE. All cell-state math in
    bf16 so DVE tensor_tensor ops run in 2x_1p mode.
  - Phase B: layer-1 forward, 2 sweeps quadrant-packed per matmul pair.
  - Phase C: layer-1 backward, only t=29 (first step of reverse scan is
    all the FC needs).
  - FC on device ([2, 512] psum per sweep); bias+transpose on host.

Scheduling: the cell tail (g-fix, c update, tanh, h) is issued one
pipeline slot late so the strict-FIFO ACT engine streams
sigma(s), tanh(s-1), sigma(s+1), ... without stalling on the DVE chain;
the Tile scheduler's PE cost is halved during build so its model doesn't
serialize around a phantom PE bottleneck (see _build_nc). DMA queues are
split by role (Sync=x, GpSimd=h1 stores, Scalar=weights+init) so the
scan's WAR chain never queues behind bulk transfers.
"""

import numpy as np
import ml_dtypes

import concourse.bass as bass
import concourse.tile as tile
from concourse import bacc, mybir
from concourse.bass_utils import run_bass_kernel_spmd

BF16 = mybir.dt.bfloat16
F32 = mybir.dt.float32
AF = mybir.ActivationFunctionType
ALU = mybir.AluOpType

H = 50
DIN = 207
DK = 208          # D + ones row
T = 30
B = 16384
NCORES = 8
BC = B // NCORES  # 2048
NSW = 4           # sweeps per core
BS = 512          # sweep batch size
Q = 64            # quadrant offset for second lane (dir-b / sweep s2)
KH = 114          # K rows for rec matmuls / h span (0:50 real, 64:114 real)
KH1 = 115         # K rows for L1 proj (incl. ones row at 114)

# PyTorch LSTM gate order in weight rows: i, f, g, o. Our bank order: i, f, o, g.
GATE_SLICES = [slice(0, 50), slice(50, 100), slice(150, 200), slice(100, 150)]
# bank 3 (g) weight cols are scaled x2: tanh(x) = 2*sigmoid(2x) - 1
GATE_SCALE = [1.0, 1.0, 1.0, 2.0]


def _pack_weights(inp):
    f32 = np.float32
    # ---- L0 projection (+bias via ones row), [208, 4*128] ----
    w0 = np.zeros((DK, 512), f32)
    wf = np.concatenate([inp["wih0f"], inp["b0f"][:, None]], axis=1)  # [200,208]
    wb = np.concatenate([inp["wih0b"], inp["b0b"][:, None]], axis=1)
    for c, (gs, sc) in enumerate(zip(GATE_SLICES, GATE_SCALE)):
        w0[:, c * 128 + 0:c * 128 + 50] = sc * wf[gs].T
        w0[:, c * 128 + Q:c * 128 + Q + 50] = sc * wb[gs].T
    # ---- L0 recurrent, block-diag [114, 4*128] ----
    r0 = np.zeros((KH, 512), f32)
    for c, (gs, sc) in enumerate(zip(GATE_SLICES, GATE_SCALE)):
        r0[0:50, c * 128 + 0:c * 128 + 50] = sc * inp["whh0f"][gs].T
        r0[Q:Q + 50, c * 128 + Q:c * 128 + Q + 50] = sc * inp["whh0b"][gs].T
    # ---- L1 fwd projection [115, 4*64] (rows: h1f 0:50, h1b 64:114, bias 114)
    def l1_proj(wih, b):
        w = np.zeros((KH1, 256), f32)
        for c, (gs, sc) in enumerate(zip(GATE_SLICES, GATE_SCALE)):
            w[0:50, c * 64:c * 64 + 50] = sc * wih[gs, 0:50].T
            w[Q:Q + 50, c * 64:c * 64 + 50] = sc * wih[gs, 50:100].T
            w[KH, c * 64:c * 64 + 50] = sc * b[gs]
        return w
    w1 = l1_proj(inp["wih1f"], inp["b1f"])
    w1b = l1_proj(inp["wih1b"], inp["b1b"])
    # ---- L1 recurrent, block-diag per sweep pair [114, 4*128] ----
    r1 = np.zeros((KH, 512), f32)
    for c, (gs, sc) in enumerate(zip(GATE_SLICES, GATE_SCALE)):
        r1[0:50, c * 128 + 0:c * 128 + 50] = sc * inp["whh1f"][gs].T
        r1[Q:Q + 50, c * 128 + Q:c * 128 + Q + 50] = sc * inp["whh1f"][gs].T
    # ---- FC ----
    wff = np.ascontiguousarray(inp["fc_w"][:, 0:50].T)  # [50, 2]
    wfb = np.ascontiguousarray(inp["fc_w"][:, 50:100].T)
    bf = ml_dtypes.bfloat16
    return {
        "w0hi": w0[0:128].astype(bf), "w0lo": w0[128:DK].astype(bf),
        "r0": r0.astype(bf), "w1": w1.astype(bf), "w1b": w1b.astype(bf),
        "r1": r1.astype(bf), "wff": wff.astype(bf), "wfb": wfb.astype(bf),
    }


def _pack_x(x):
    # x [B, T, 207] f32 -> [T, 208, B] bf16 with ones row at d=207
    xt = np.empty((T, DK, B), dtype=ml_dtypes.bfloat16)
    xt[:, 0:DIN, :] = x.transpose(1, 2, 0).astype(ml_dtypes.bfloat16)
    xt[:, DIN, :] = ml_dtypes.bfloat16(1.0)
    return xt


def _make_hinit():
    # h1-history fixed rows, DMA'd instead of memset: row 0 -> ones row
    # (partition 114, the L1 bias row), rows 1:15 -> zeros (partitions
    # 50:64, the dead K rows between the fwd and bwd quadrants).
    hi = np.zeros((15, T, BS), dtype=ml_dtypes.bfloat16)
    hi[0] = ml_dtypes.bfloat16(1.0)
    return hi


def _build_nc():
    # The Tile scheduler's cost model times each matmul as N serial cycles,
    # but our proj matmuls run pairwise-concurrent on disjoint PE col
    # groups (quadrant packing), so the model overestimates PE time ~2x.
    # That mis-model makes the scheduler serialize the ACT stream around a
    # phantom PE bottleneck (sigma -> DVE -> tanh -> sigma chains). Halve
    # the model's PE cycle during scheduling so instruction ORDER is chosen
    # around the real ACT bottleneck; semaphores still enforce the true
    # data dependencies, so this only affects heuristic ordering.
    from concourse import hw_specs
    spec = hw_specs.TRN2Spec
    saved = (spec.PE_CYCLE, spec.PE_CYCLE_PSTATE_MID, spec.PE_CYCLE_PSTATE_LOW)
    spec.PE_CYCLE = saved[0] * 0.5
    spec.PE_CYCLE_PSTATE_MID = saved[1] * 0.5
    spec.PE_CYCLE_PSTATE_LOW = saved[2] * 0.5
    try:
        return _build_nc_inner()
    finally:
        (spec.PE_CYCLE, spec.PE_CYCLE_PSTATE_MID,
         spec.PE_CYCLE_PSTATE_LOW) = saved


def _build_nc_inner():
    nc = bacc.Bacc("TRN2", target_bir_lowering=False, debug=False)
    ap = {}
    ap["xT"] = nc.dram_tensor("xT", [T, DK, BC], BF16, kind="ExternalInput").ap()
    for name, shp in [("w0hi", [128, 512]), ("w0lo", [DK - 128, 512]),
                      ("r0", [KH, 512]), ("w1", [KH1, 256]), ("w1b", [KH1, 256]),
                      ("r1", [KH, 512]), ("wff", [50, 2]), ("wfb", [50, 2]),
                      ("hinit", [15, T, BS])]:
        ap[name] = nc.dram_tensor(name, shp, BF16, kind="ExternalInput").ap()
    out_ap = nc.dram_tensor("out", [2, BC], F32, kind="ExternalOutput").ap()

    with tile.TileContext(nc) as tc:
        with (
            tc.tile_pool(name="wts", bufs=1) as wp,
            tc.tile_pool(name="xin", bufs=2) as xp,
            tc.tile_pool(name="h1p", bufs=1) as h1p,
            tc.tile_pool(name="st", bufs=1) as sp,
            tc.tile_pool(name="gt", bufs=2) as gp,
            tc.tile_pool(name="outp", bufs=2) as op_,
        ):
            # ---- load weights to SBUF ----
            # DMA queue assignment: Sync = x loads only, GpSimd = h1 stores
            # only (they gate the scan's WAR chain, so nothing may queue
            # ahead of them), Scalar = all weights + h1 fixed-row init
            # (Vector cannot issue DMAs). This keeps Phase A's first matmul and first
            # h1 stores off the 2.4MB of cold-start transfers.
            w = {}
            for name, shp in [("w0hi", [128, 512]), ("w0lo", [DK - 128, 512]),
                              ("r0", [KH, 512]), ("w1", [KH1, 256]),
                              ("w1b", [KH1, 256]), ("r1", [KH, 512]),
                              ("wff", [50, 2]), ("wfb", [50, 2])]:
                t = wp.tile(shp, BF16, tag=name, name=name)
                nc.scalar.dma_start(t[:], ap[name][:])
                w[name] = t
            # FC weights staged at both quadrants (rhs base-partition match)
            for name in ("wff", "wfb"):
                t = wp.tile([128, 2], BF16, tag=name + "q", name=name + "q")
                nc.scalar.dma_start(t[0:50, :], ap[name][:])
                nc.scalar.dma_start(t[Q:Q + 50, :], ap[name][:])
                w[name + "q"] = t

            # ---- h1 history per sweep [115, T, 512]; row 114 = ones ----
            # Fixed rows come in via DMA (memsets of these ranges cost
            # ~100us of engine time and serialized the kernel start).
            h1 = []
            for s in range(NSW):
                t = h1p.tile([128, T, BS], BF16, tag=f"h1_{s}", name=f"h1_{s}")
                nc.scalar.dma_start(t[114:115, :, :], ap["hinit"][0:1, :, :])
                # rows 50:64 are never written by the scan; zero them so the
                # 0-weight K rows in layer-1 matmuls read 0 (not NaN).
                nc.scalar.dma_start(t[50:64, :, :], ap["hinit"][1:15, :, :])
                h1.append(t)

            # ---- states (bf16 cell state: DVE 2x_1p mode) ----
            hS = [sp.tile([128, BS], BF16, tag=f"hs{s}", name=f"hs{s}") for s in range(NSW)]
            cS = [sp.tile([128, BS], BF16, tag=f"cs{s}", name=f"cs{s}") for s in range(NSW)]
            hB = [sp.tile([128, BS], BF16, tag=f"hb{p}", name=f"hb{p}") for p in range(2)]
            cB = [sp.tile([128, BS], BF16, tag=f"cb{p}", name=f"cb{p}") for p in range(2)]
            hC = [sp.tile([128, BS], BF16, tag=f"hc{p}", name=f"hc{p}") for p in range(2)]
            cC = [sp.tile([128, BS], BF16, tag=f"cc{p}", name=f"cc{p}") for p in range(2)]

            def cell_p1(P, G):
                """sigmoid over all 4 banks (g weights pre-scaled x2: bank 3
                holds sig(2*ag), converted to tanh in cell_p2). Everything
                after the sigmoid runs one pipeline slot later (cell_p2):
                the next sweep's sigma then overlaps this sweep's DVE chain,
                and when its tanh reaches the strict-FIFO ACT head the cell
                state is already updated -- the ACT engine never stalls."""
                nc.scalar.activation(G[0:KH, :, :], P[0:KH, :, :], AF.Sigmoid)

            def cell_c(G, c_t, k):
                """g = 2*sig-1 and the c update (no tanh/h yet)."""
                nc.vector.tensor_scalar(G[0:KH, 3, :], G[0:KH, 3, :],
                                        2.0, -1.0, ALU.mult, ALU.add)
                i_, f_, g_ = G[0:KH, 0, :], G[0:KH, 1, :], G[0:KH, 3, :]
                if k == 0:
                    nc.vector.tensor_mul(c_t[0:KH, :], i_, g_)
                else:
                    nc.vector.tensor_mul(g_, i_, g_)
                    nc.vector.tensor_mul(c_t[0:KH, :], f_, c_t[0:KH, :])
                    nc.vector.tensor_add(c_t[0:KH, :], c_t[0:KH, :], g_)

            def cell_p2(G, c_t, h_out, k):
                """full cell tail: c update, tanh(c) into G slot 0 (i dead
                after the ig product), h = o * th. Used by Phases B/C."""
                cell_c(G, c_t, k)
                th = G[0:KH, 0, :]
                nc.scalar.activation(th, c_t[0:KH, :], AF.Tanh)
                nc.vector.tensor_mul(h_out[0:KH, :], G[0:KH, 2, :], th)

            with tc.tile_pool(name="ps", bufs=2, space="PSUM") as pp:
                # =================== Phase A: layer 0, lockstep ===========
                pend = None  # (G, s, k) whose cell tail runs next slot

                def flush_a():
                    nonlocal pend
                    Gp, sp, kp = pend
                    cell_p2(Gp, cS[sp], hS[sp], kp)
                    # h1 history is time-aligned: bwd lane at step kp holds
                    # h_b for time T-1-kp. Stores go on the GpSimd queue so
                    # the Sync queue only feeds x loads.
                    nc.gpsimd.dma_start(h1[sp][0:50, kp, :], hS[sp][0:50, :])
                    nc.gpsimd.dma_start(h1[sp][Q:Q + 50, T - 1 - kp, :],
                                        hS[sp][Q:Q + 50, :])
                    pend = None

                for k in range(T):
                  for half in range(2):
                    # one DMA per (K-chunk, direction) covering 2 sweeps
                    hb = bass.ds(half * 2 * BS, 2 * BS)
                    xfh = xp.tile([128, 2 * BS], BF16, tag="xfh", name="xfh")
                    nc.sync.dma_start(xfh[:], ap["xT"][k, 0:128, hb])
                    xfl = xp.tile([DK - 128, 2 * BS], BF16, tag="xfl", name="xfl")
                    nc.sync.dma_start(xfl[:], ap["xT"][k, 128:DK, hb])
                    xbh = xp.tile([128, 2 * BS], BF16, tag="xbh", name="xbh")
                    nc.sync.dma_start(xbh[:], ap["xT"][T - 1 - k, 0:128, hb])
                    xbl = xp.tile([DK - 128, 2 * BS], BF16, tag="xbl", name="xbl")
                    nc.sync.dma_start(xbl[:], ap["xT"][T - 1 - k, 128:DK, hb])

                    for s in (2 * half, 2 * half + 1):
                        sl = bass.ts(s % 2, BS)
                        # Rotated interleave: adjacent MMs hit disjoint
                        # col-groups (concurrent) while same-bank writes are
                        # >=2 slots apart (concurrent drains from different
                        # col-groups into one PSUM bank corrupt it).
                        P = pp.tile([128, 4, BS], F32, tag="ps", name="ps")
                        for wt_, xfx, xbx, st in ((w["w0hi"], xfh, xbh, True),
                                                  (w["w0lo"], xfl, xbl, False)):
                            for c in range(4):
                                cb = (c + 2) % 4
                                spA = k == 0 and not st and c >= 2
                                spB = k == 0 and not st and cb < 2
                                nc.tensor.matmul(
                                    P[0:Q, c, :],
                                    wt_[:, bass.ds(c * 128, Q)], xfx[:, sl],
                                    start=st, stop=spA)
                                nc.tensor.matmul(
                                    P[Q:128, cb, :],
                                    wt_[:, bass.ds(cb * 128 + Q, Q)], xbx[:, sl],
                                    start=st, stop=spB)
                        if k > 0:
                            for c in range(4):
                                nc.tensor.matmul(
                                    P[:, c, :],
                                    w["r0"][:, bass.ts(c, 128)], hS[s][0:KH, :],
                                    start=False, stop=True)

                        G = gp.tile([128, 4, BS], BF16, tag=f"g{s}", name=f"g{s}")
                        cell_p1(P, G)
                        if pend is not None:
                            flush_a()
                        pend = (G, s, k)
                flush_a()

                # ============ Phase C: layer 1 backward (t=29 only) =======
                # Independent of Phase B (the reverse scan's first step needs
                # only h1[T-1], complete once Phase A ends) -- issued before B
                # so the scheduler can fill B's pipeline-warmup bubbles with
                # it instead of paying for it serially at the tail.
                gC = []
                for p in range(2):
                    s1, s2 = 2 * p, 2 * p + 1
                    P = pp.tile([128, 4, BS], F32, tag="ps", name="ps")
                    for c in range(4):
                        cb = (c + 2) % 4
                        nc.tensor.matmul(
                            P[0:Q, c, :], w["w1b"][:, bass.ts(c, 64)],
                            h1[s1][0:KH1, T - 1, :], start=True, stop=c >= 2)
                        nc.tensor.matmul(
                            P[Q:128, cb, :], w["w1b"][:, bass.ts(cb, 64)],
                            h1[s2][0:KH1, T - 1, :], start=True, stop=cb < 2)
                    G = gp.tile([128, 4, BS], BF16, tag=f"g{p + 2}", name=f"g{p + 2}")
                    cell_p1(P, G)
                    gC.append(G)
                for p in range(2):
                    cell_p2(gC[p], cC[p], hC[p], 0)

                # =================== Phase B: layer 1 forward =============
                pendb = None  # (G, p, k)

                def flush_b():
                    nonlocal pendb
                    Gp, pp_, kp = pendb
                    cell_p2(Gp, cB[pp_], hB[pp_], kp)
                    pendb = None

                for k in range(T):
                    for p in range(2):
                        s1, s2 = 2 * p, 2 * p + 1
                        P = pp.tile([128, 4, BS], F32, tag="ps", name="ps")
                        for c in range(4):
                            cb = (c + 2) % 4
                            nc.tensor.matmul(
                                P[0:Q, c, :], w["w1"][:, bass.ts(c, 64)],
                                h1[s1][0:KH1, k, :], start=True,
                                stop=(k == 0 and c >= 2))
                            nc.tensor.matmul(
                                P[Q:128, cb, :], w["w1"][:, bass.ts(cb, 64)],
                                h1[s2][0:KH1, k, :], start=True,
                                stop=(k == 0 and cb < 2))
                        if k > 0:
                            for c in range(4):
                                nc.tensor.matmul(
                                    P[:, c, :],
                                    w["r1"][:, bass.ts(c, 128)], hB[p][0:KH, :],
                                    start=False, stop=True)
                        G = gp.tile([128, 4, BS], BF16, tag=f"g{p}", name=f"g{p}")
                        cell_p1(P, G)
                        if pendb is not None:
                            flush_b()
                        pendb = (G, p, k)
                flush_b()

            # =================== FC ====================================
            with tc.tile_pool(name="fcp", bufs=4, space="PSUM") as fcp:
                for s in range(NSW):
                    p, qo = s // 2, (s % 2) * Q
                    F = fcp.tile([2, BS], F32, tag="fc", name="fc")
                    nc.tensor.matmul(F[:], w["wffq"][qo:qo + 50, :],
                                     hB[p][qo:qo + 50, :], start=True, stop=False)
                    nc.tensor.matmul(F[:], w["wfbq"][qo:qo + 50, :],
                                     hC[p][qo:qo + 50, :], start=False, stop=True)
                    ot = op_.tile([2, BS], F32, tag="o", name="ot")
                    nc.vector.tensor_copy(ot[:], F[:])
                    nc.sync.dma_start(out_ap[:, bass.ts(s, BS)], ot[:])

    nc.compile()
    return nc


_NC_CACHE = None


def kernel(**inputs) -> np.ndarray:
    global _NC_CACHE
    if _NC_CACHE is None:
        _NC_CACHE = _build_nc()
    nc = _NC_CACHE
    wts = _pack_weights(inputs)
    wts["hinit"] = _make_hinit()
    xt = _pack_x(np.asarray(inputs["x"], dtype=np.float32))
    in_maps = []
    for c in range(NCORES):
        m = dict(wts)
        m["xT"] = np.ascontiguousarray(xt[:, :, c * BC:(c + 1) * BC])
        in_maps.append(m)
    res = run_bass_kernel_spmd(nc, in_maps, list(range(NCORES)))
    outs = [res.results[c]["out"] for c in range(NCORES)]  # [2, BC] each
    full = np.concatenate(outs, axis=1).T  # [B, 2]
    return (full + inputs["fc_b"][None, :]).astype(np.float32)


# revision 32
# speedup vs baseline: 1.0156x; 1.0156x over previous
"""BiLSTM (2-layer, H=50, D=207, T=30, B=16384) -> FC(2) Trainium2 kernel.

Data-parallel over 8 NeuronCores (2048 batch rows each). Host pre-packs
x into feature-major bf16 layout [T, 208, B] (ones row folds biases into
the input projection) and reorders weights into PSUM-bank gate layout.

Device layout (per core, 4 sweeps of 512 batch):
  - quadrant packing: partitions 0:50 = dir-f (or sweep s1), 64:114 =
    dir-b (or sweep s2); weight columns zero-padded so junk rows are 0.
  - Phase A: layer-0 fwd+bwd scans in lockstep (fwd eats x[t=k], bwd
    x[t=29-k]); gates accumulate in PSUM [128, 4banks]; ONE sigmoid
    ACTIVATE over all 4 banks -- the g-gate weight columns are
    pre-scaled x2 on host so tanh(ag) = 2*sigmoid(2ag)-1, recovered
    with a single 4x-mode tensor_scalar on DVE. All cell-state math in
    bf16 so DVE tensor_tensor ops run in 2x_1p mode.
  - Phase B: layer-1 forward, 2 sweeps quadrant-packed per matmul pair.
  - Phase C: layer-1 backward, only t=29 (first step of reverse scan is
    all the FC needs).
  - FC on device ([2, 512] psum per sweep); bias+transpose on host.

Scheduling: the cell tail (g-fix, c update, tanh, h) is issued one
pipeline slot late so the strict-FIFO ACT engine streams
sigma(s), tanh(s-1), sigma(s+1), ... without stalling on the DVE chain;
the Tile scheduler's PE cost is halved during build so its model doesn't
serialize around a phantom PE bottleneck (see _build_nc). DMA queues are
split by role (Sync=x, GpSimd=h1 stores, Scalar=weights+init) so the
scan's WAR chain never queues behind bulk transfers.
"""

import numpy as np
import ml_dtypes

import concourse.bass as bass
import concourse.tile as tile
from concourse import bacc, mybir
from concourse.bass_utils import run_bass_kernel_spmd

BF16 = mybir.dt.bfloat16
F32 = mybir.dt.float32
AF = mybir.ActivationFunctionType
ALU = mybir.AluOpType

H = 50
DIN = 207
DK = 208          # D + ones row
T = 30
B = 16384
NCORES = 8
BC = B // NCORES  # 2048
NSW = 4           # sweeps per core
BS = 512          # sweep batch size
Q = 64            # quadrant offset for second lane (dir-b / sweep s2)
KH = 114          # K rows for rec matmuls / h span (0:50 real, 64:114 real)
KH1 = 115         # K rows for L1 proj (incl. ones row at 114)

# PyTorch LSTM gate order in weight rows: i, f, g, o. Our bank order: i, f, o, g.
GATE_SLICES = [slice(0, 50), slice(50, 100), slice(150, 200), slice(100, 150)]
# bank 3 (g) weight cols are scaled x2: tanh(x) = 2*sigmoid(2x) - 1
GATE_SCALE = [1.0, 1.0, 1.0, 2.0]


def _pack_weights(inp):
    f32 = np.float32
    # ---- L0 projection (+bias via ones row), [208, 4*128] ----
    w0 = np.zeros((DK, 512), f32)
    wf = np.concatenate([inp["wih0f"], inp["b0f"][:, None]], axis=1)  # [200,208]
    wb = np.concatenate([inp["wih0b"], inp["b0b"][:, None]], axis=1)
    for c, (gs, sc) in enumerate(zip(GATE_SLICES, GATE_SCALE)):
        w0[:, c * 128 + 0:c * 128 + 50] = sc * wf[gs].T
        w0[:, c * 128 + Q:c * 128 + Q + 50] = sc * wb[gs].T
    # ---- L0 recurrent, block-diag [114, 4*128] ----
    r0 = np.zeros((KH, 512), f32)
    for c, (gs, sc) in enumerate(zip(GATE_SLICES, GATE_SCALE)):
        r0[0:50, c * 128 + 0:c * 128 + 50] = sc * inp["whh0f"][gs].T
        r0[Q:Q + 50, c * 128 + Q:c * 128 + Q + 50] = sc * inp["whh0b"][gs].T
    # ---- L1 fwd projection [115, 4*64] (rows: h1f 0:50, h1b 64:114, bias 114)
    def l1_proj(wih, b):
        w = np.zeros((KH1, 256), f32)
        for c, (gs, sc) in enumerate(zip(GATE_SLICES, GATE_SCALE)):
            w[0:50, c * 64:c * 64 + 50] = sc * wih[gs, 0:50].T
            w[Q:Q + 50, c * 64:c * 64 + 50] = sc * wih[gs, 50:100].T
            w[KH, c * 64:c * 64 + 50] = sc * b[gs]
        return w
    w1 = l1_proj(inp["wih1f"], inp["b1f"])
    w1b = l1_proj(inp["wih1b"], inp["b1b"])
    # ---- L1 recurrent, block-diag per sweep pair [114, 4*128] ----
    r1 = np.zeros((KH, 512), f32)
    for c, (gs, sc) in enumerate(zip(GATE_SLICES, GATE_SCALE)):
        r1[0:50, c * 128 + 0:c * 128 + 50] = sc * inp["whh1f"][gs].T
        r1[Q:Q + 50, c * 128 + Q:c * 128 + Q + 50] = sc * inp["whh1f"][gs].T
    # ---- FC ----
    wff = np.ascontiguousarray(inp["fc_w"][:, 0:50].T)  # [50, 2]
    wfb = np.ascontiguousarray(inp["fc_w"][:, 50:100].T)
    bf = ml_dtypes.bfloat16
    return {
        "w0hi": w0[0:128].astype(bf), "w0lo": w0[128:DK].astype(bf),
        "r0": r0.astype(bf), "w1": w1.astype(bf), "w1b": w1b.astype(bf),
        "r1": r1.astype(bf), "wff": wff.astype(bf), "wfb": wfb.astype(bf),
    }


def _pack_x(x):
    # x [B, T, 207] f32 -> [T, 208, B] bf16 with ones row at d=207
    xt = np.empty((T, DK, B), dtype=ml_dtypes.bfloat16)
    xt[:, 0:DIN, :] = x.transpose(1, 2, 0).astype(ml_dtypes.bfloat16)
    xt[:, DIN, :] = ml_dtypes.bfloat16(1.0)
    return xt


def _make_hinit():
    # h1-history fixed rows, DMA'd instead of memset: row 0 -> ones row
    # (partition 114, the L1 bias row), rows 1:15 -> zeros (partitions
    # 50:64, the dead K rows between the fwd and bwd quadrants).
    hi = np.zeros((15, T, BS), dtype=ml_dtypes.bfloat16)
    hi[0] = ml_dtypes.bfloat16(1.0)
    return hi


def _build_nc():
    # The Tile scheduler's cost model times each matmul as N serial cycles,
    # but our proj matmuls run pairwise-concurrent on disjoint PE col
    # groups (quadrant packing), so the model overestimates PE time ~2x.
    # That mis-model makes the scheduler serialize the ACT stream around a
    # phantom PE bottleneck (sigma -> DVE -> tanh -> sigma chains). Halve
    # the model's PE cycle during scheduling so instruction ORDER is chosen
    # around the real ACT bottleneck; semaphores still enforce the true
    # data dependencies, so this only affects heuristic ordering.
    from concourse import hw_specs
    spec = hw_specs.TRN2Spec
    saved = (spec.PE_CYCLE, spec.PE_CYCLE_PSTATE_MID, spec.PE_CYCLE_PSTATE_LOW)
    spec.PE_CYCLE = saved[0] * 0.5
    spec.PE_CYCLE_PSTATE_MID = saved[1] * 0.5
    spec.PE_CYCLE_PSTATE_LOW = saved[2] * 0.5
    try:
        return _build_nc_inner()
    finally:
        (spec.PE_CYCLE, spec.PE_CYCLE_PSTATE_MID,
         spec.PE_CYCLE_PSTATE_LOW) = saved


def _build_nc_inner():
    nc = bacc.Bacc("TRN2", target_bir_lowering=False, debug=False)
    ap = {}
    ap["xT"] = nc.dram_tensor("xT", [T, DK, BC], BF16, kind="ExternalInput").ap()
    for name, shp in [("w0hi", [128, 512]), ("w0lo", [DK - 128, 512]),
                      ("r0", [KH, 512]), ("w1", [KH1, 256]), ("w1b", [KH1, 256]),
                      ("r1", [KH, 512]), ("wff", [50, 2]), ("wfb", [50, 2]),
                      ("hinit", [15, T, BS])]:
        ap[name] = nc.dram_tensor(name, shp, BF16, kind="ExternalInput").ap()
    out_ap = nc.dram_tensor("out", [2, BC], F32, kind="ExternalOutput").ap()

    with tile.TileContext(nc) as tc:
        with (
            tc.tile_pool(name="wts", bufs=1) as wp,
            tc.tile_pool(name="xin", bufs=2) as xp,
            tc.tile_pool(name="h1p", bufs=1) as h1p,
            tc.tile_pool(name="st", bufs=1) as sp,
            tc.tile_pool(name="gt", bufs=2) as gp,
            tc.tile_pool(name="outp", bufs=2) as op_,
        ):
            # ---- load weights to SBUF ----
            # DMA queue assignment: Sync = x loads, GpSimd = h1 stores only
            # (they gate the scan's WAR chain, so nothing may queue ahead of
            # them), Scalar = the three Phase-A weights. Everything not
            # needed until Phase B/C (L1 weights, FC weights, h1 fixed-row
            # init) is deferred into the Phase A loop on the Sync queue --
            # with all 20 issues up front the first sigma couldn't reach the
            # ACT queue head until ~31us.
            w = {}
            for name, shp in [("w0hi", [128, 512]), ("w0lo", [DK - 128, 512]),
                              ("r0", [KH, 512]), ("w1", [KH1, 256]),
                              ("w1b", [KH1, 256]), ("r1", [KH, 512]),
                              ("wff", [50, 2]), ("wfb", [50, 2])]:
                t = wp.tile(shp, BF16, tag=name, name=name)
                if name in ("w0hi", "w0lo", "r0"):
                    nc.scalar.dma_start(t[:], ap[name][:])
                else:
                    pass  # deferred below
                w[name] = t
            # FC weights staged at both quadrants (rhs base-partition match)
            for name in ("wff", "wfb"):
                w[name + "q"] = wp.tile([128, 2], BF16, tag=name + "q",
                                        name=name + "q")

            # ---- h1 history per sweep [115, T, 512]; row 114 = ones ----
            # Fixed rows come in via DMA (memsets of these ranges cost
            # ~100us of engine time and serialized the kernel start).
            h1 = [h1p.tile([128, T, BS], BF16, tag=f"h1_{s}", name=f"h1_{s}")
                  for s in range(NSW)]

            deferred = [(w["w1"][:], ap["w1"][:]), (w["w1b"][:], ap["w1b"][:]),
                        (w["r1"][:], ap["r1"][:]), (w["wff"][:], ap["wff"][:]),
                        (w["wfb"][:], ap["wfb"][:])]
            for name in ("wff", "wfb"):
                deferred.append((w[name + "q"][0:50, :], ap[name][:]))
                deferred.append((w[name + "q"][Q:Q + 50, :], ap[name][:]))
            for s in range(NSW):
                deferred.append((h1[s][114:115, :, :], ap["hinit"][0:1, :, :]))
                # rows 50:64 are never written by the scan; zero them so the
                # 0-weight K rows in layer-1 matmuls read 0 (not NaN).
                deferred.append((h1[s][50:64, :, :], ap["hinit"][1:15, :, :]))

            # ---- states (bf16 cell state: DVE 2x_1p mode) ----
            hS = [sp.tile([128, BS], BF16, tag=f"hs{s}", name=f"hs{s}") for s in range(NSW)]
            cS = [sp.tile([128, BS], BF16, tag=f"cs{s}", name=f"cs{s}") for s in range(NSW)]
            hB = [sp.tile([128, BS], BF16, tag=f"hb{p}", name=f"hb{p}") for p in range(2)]
            cB = [sp.tile([128, BS], BF16, tag=f"cb{p}", name=f"cb{p}") for p in range(2)]
            hC = [sp.tile([128, BS], BF16, tag=f"hc{p}", name=f"hc{p}") for p in range(2)]
            cC = [sp.tile([128, BS], BF16, tag=f"cc{p}", name=f"cc{p}") for p in range(2)]

            def cell_p1(P, G):
                """sigmoid over all 4 banks (g weights pre-scaled x2: bank 3
                holds sig(2*ag), converted to tanh in cell_p2). Everything
                after the sigmoid runs one pipeline slot later (cell_p2):
                the next sweep's sigma then overlaps this sweep's DVE chain,
                and when its tanh reaches the strict-FIFO ACT head the cell
                state is already updated -- the ACT engine never stalls."""
                nc.scalar.activation(G[0:KH, :, :], P[0:KH, :, :], AF.Sigmoid)

            def cell_c(G, c_t, k):
                """g = 2*sig-1 and the c update (no tanh/h yet)."""
                nc.vector.tensor_scalar(G[0:KH, 3, :], G[0:KH, 3, :],
                                        2.0, -1.0, ALU.mult, ALU.add)
                i_, f_, g_ = G[0:KH, 0, :], G[0:KH, 1, :], G[0:KH, 3, :]
                if k == 0:
                    nc.vector.tensor_mul(c_t[0:KH, :], i_, g_)
                else:
                    nc.vector.tensor_mul(g_, i_, g_)
                    nc.vector.tensor_mul(c_t[0:KH, :], f_, c_t[0:KH, :])
                    nc.vector.tensor_add(c_t[0:KH, :], c_t[0:KH, :], g_)

            def cell_p2(G, c_t, h_out, k):
                """full cell tail: c update, tanh(c) into G slot 0 (i dead
                after the ig product), h = o * th. Used by Phases B/C."""
                cell_c(G, c_t, k)
                th = G[0:KH, 0, :]
                nc.scalar.activation(th, c_t[0:KH, :], AF.Tanh)
                nc.vector.tensor_mul(h_out[0:KH, :], G[0:KH, 2, :], th)

            with tc.tile_pool(name="ps", bufs=2, space="PSUM") as pp:
                # =================== Phase A: layer 0, lockstep ===========
                pend = None  # (G, s, k) whose cell tail runs next slot

                def flush_a():
                    nonlocal pend
                    Gp, sp, kp = pend
                    cell_p2(Gp, cS[sp], hS[sp], kp)
                    # h1 history is time-aligned: bwd lane at step kp holds
                    # h_b for time T-1-kp. Stores go on the GpSimd queue so
                    # the Sync queue only feeds x loads.
                    nc.gpsimd.dma_start(h1[sp][0:50, kp, :], hS[sp][0:50, :])
                    nc.gpsimd.dma_start(h1[sp][Q:Q + 50, T - 1 - kp, :],
                                        hS[sp][Q:Q + 50, :])
                    pend = None

                for k in range(T):
                  for half in range(2):
                    # trickle out the Phase-B/C weight + h1-init transfers
                    # once the scan pipeline is running
                    if 2 <= k <= 5 and half == 1:
                        for _ in range(5):
                            if deferred:
                                dst, src = deferred.pop(0)
                                nc.sync.dma_start(dst, src)
                    # one DMA per (K-chunk, direction) covering 2 sweeps
                    hb = bass.ds(half * 2 * BS, 2 * BS)
                    xfh = xp.tile([128, 2 * BS], BF16, tag="xfh", name="xfh")
                    nc.sync.dma_start(xfh[:], ap["xT"][k, 0:128, hb])
                    xfl = xp.tile([DK - 128, 2 * BS], BF16, tag="xfl", name="xfl")
                    nc.sync.dma_start(xfl[:], ap["xT"][k, 128:DK, hb])
                    xbh = xp.tile([128, 2 * BS], BF16, tag="xbh", name="xbh")
                    nc.sync.dma_start(xbh[:], ap["xT"][T - 1 - k, 0:128, hb])
                    xbl = xp.tile([DK - 128, 2 * BS], BF16, tag="xbl", name="xbl")
                    nc.sync.dma_start(xbl[:], ap["xT"][T - 1 - k, 128:DK, hb])

                    for s in (2 * half, 2 * half + 1):
                        sl = bass.ts(s % 2, BS)
                        # Rotated interleave: adjacent MMs hit disjoint
                        # col-groups (concurrent) while same-bank writes are
                        # >=2 slots apart (concurrent drains from different
                        # col-groups into one PSUM bank corrupt it).
                        P = pp.tile([128, 4, BS], F32, tag="ps", name="ps")
                        for wt_, xfx, xbx, st in ((w["w0hi"], xfh, xbh, True),
                                                  (w["w0lo"], xfl, xbl, False)):
                            for c in range(4):
                                cb = (c + 2) % 4
                                spA = k == 0 and not st and c >= 2
                                spB = k == 0 and not st and cb < 2
                                nc.tensor.matmul(
                                    P[0:Q, c, :],
                                    wt_[:, bass.ds(c * 128, Q)], xfx[:, sl],
                                    start=st, stop=spA)
                                nc.tensor.matmul(
                                    P[Q:128, cb, :],
                                    wt_[:, bass.ds(cb * 128 + Q, Q)], xbx[:, sl],
                                    start=st, stop=spB)
                        if k > 0:
                            for c in range(4):
                                nc.tensor.matmul(
                                    P[:, c, :],
                                    w["r0"][:, bass.ts(c, 128)], hS[s][0:KH, :],
                                    start=False, stop=True)

                        G = gp.tile([128, 4, BS], BF16, tag=f"g{s}", name=f"g{s}")
                        cell_p1(P, G)
                        if pend is not None:
                            flush_a()
                        pend = (G, s, k)
                flush_a()

                # ============ Phase C: layer 1 backward (t=29 only) =======
                # Independent of Phase B (the reverse scan's first step needs
                # only h1[T-1], complete once Phase A ends) -- issued before B
                # so the scheduler can fill B's pipeline-warmup bubbles with
                # it instead of paying for it serially at the tail.
                gC = []
                for p in range(2):
                    s1, s2 = 2 * p, 2 * p + 1
                    P = pp.tile([128, 4, BS], F32, tag="ps", name="ps")
                    for c in range(4):
                        cb = (c + 2) % 4
                        nc.tensor.matmul(
                            P[0:Q, c, :], w["w1b"][:, bass.ts(c, 64)],
                            h1[s1][0:KH1, T - 1, :], start=True, stop=c >= 2)
                        nc.tensor.matmul(
                            P[Q:128, cb, :], w["w1b"][:, bass.ts(cb, 64)],
                            h1[s2][0:KH1, T - 1, :], start=True, stop=cb < 2)
                    G = gp.tile([128, 4, BS], BF16, tag=f"g{p + 2}", name=f"g{p + 2}")
                    cell_p1(P, G)
                    gC.append(G)
                for p in range(2):
                    cell_p2(gC[p], cC[p], hC[p], 0)

                # =================== Phase B: layer 1 forward =============
                pendb = None  # (G, p, k)

                def flush_b():
                    nonlocal pendb
                    Gp, pp_, kp = pendb
                    cell_p2(Gp, cB[pp_], hB[pp_], kp)
                    pendb = None

                for k in range(T):
                    for p in range(2):
                        s1, s2 = 2 * p, 2 * p + 1
                        P = pp.tile([128, 4, BS], F32, tag="ps", name="ps")
                        for c in range(4):
                            cb = (c + 2) % 4
                            nc.tensor.matmul(
                                P[0:Q, c, :], w["w1"][:, bass.ts(c, 64)],
                                h1[s1][0:KH1, k, :], start=True,
                                stop=(k == 0 and c >= 2))
                            nc.tensor.matmul(
                                P[Q:128, cb, :], w["w1"][:, bass.ts(cb, 64)],
                                h1[s2][0:KH1, k, :], start=True,
                                stop=(k == 0 and cb < 2))
                        if k > 0:
                            for c in range(4):
                                nc.tensor.matmul(
                                    P[:, c, :],
                                    w["r1"][:, bass.ts(c, 128)], hB[p][0:KH, :],
                                    start=False, stop=True)
                        G = gp.tile([128, 4, BS], BF16, tag=f"g{p}", name=f"g{p}")
                        cell_p1(P, G)
                        if pendb is not None:
                            flush_b()
                        pendb = (G, p, k)
                flush_b()

            # =================== FC ====================================
            with tc.tile_pool(name="fcp", bufs=4, space="PSUM") as fcp:
                for s in range(NSW):
                    p, qo = s // 2, (s % 2) * Q
                    F = fcp.tile([2, BS], F32, tag="fc", name="fc")
                    nc.tensor.matmul(F[:], w["wffq"][qo:qo + 50, :],
                                     hB[p][qo:qo + 50, :], start=True, stop=False)
                    nc.tensor.matmul(F[:], w["wfbq"][qo:qo + 50, :],
                                     hC[p][qo:qo + 50, :], start=False, stop=True)
                    ot = op_.tile([2, BS], F32, tag="o", name="ot")
                    nc.vector.tensor_copy(ot[:], F[:])
                    nc.sync.dma_start(out_ap[:, bass.ts(s, BS)], ot[:])

    nc.compile()
    return nc


_NC_CACHE = None


def kernel(**inputs) -> np.ndarray:
    global _NC_CACHE
    if _NC_CACHE is None:
        _NC_CACHE = _build_nc()
    nc = _NC_CACHE
    wts = _pack_weights(inputs)
    wts["hinit"] = _make_hinit()
    xt = _pack_x(np.asarray(inputs["x"], dtype=np.float32))
    in_maps = []
    for c in range(NCORES):
        m = dict(wts)
        m["xT"] = np.ascontiguousarray(xt[:, :, c * BC:(c + 1) * BC])
        in_maps.append(m)
    res = run_bass_kernel_spmd(nc, in_maps, list(range(NCORES)))
    outs = [res.results[c]["out"] for c in range(NCORES)]  # [2, BC] each
    full = np.concatenate(outs, axis=1).T  # [B, 2]
    return (full + inputs["fc_b"][None, :]).astype(np.float32)


# revision 34
# speedup vs baseline: 1.0190x; 1.0033x over previous
"""BiLSTM (2-layer, H=50, D=207, T=30, B=16384) -> FC(2) Trainium2 kernel.

Data-parallel over 8 NeuronCores (2048 batch rows each). Host pre-packs
x into feature-major bf16 layout [T, 208, B] (ones row folds biases into
the input projection) and reorders weights into PSUM-bank gate layout.

Device layout (per core, 4 sweeps of 512 batch):
  - quadrant packing: partitions 0:50 = dir-f (or sweep s1), 64:114 =
    dir-b (or sweep s2); weight columns zero-padded so junk rows are 0.
  - Phase A: layer-0 fwd+bwd scans in lockstep (fwd eats x[t=k], bwd
    x[t=29-k]); gates accumulate in PSUM [128, 4banks]; ONE sigmoid
    ACTIVATE over all 4 banks -- the g-gate weight columns are
    pre-scaled x2 on host so tanh(ag) = 2*sigmoid(2ag)-1, recovered
    with a single 4x-mode tensor_scalar on DVE. All cell-state math in
    bf16 so DVE tensor_tensor ops run in 2x_1p mode.
  - Phase B: layer-1 forward, 2 sweeps quadrant-packed per matmul pair.
  - Phase C: layer-1 backward, only t=29 (first step of reverse scan is
    all the FC needs).
  - FC on device ([2, 512] psum per sweep); bias+transpose on host.

Scheduling: the cell tail (g-fix, c update, tanh, h) is issued one
pipeline slot late so the strict-FIFO ACT engine streams
sigma(s), tanh(s-1), sigma(s+1), ... without stalling on the DVE chain;
the Tile scheduler's PE cost is halved during build so its model doesn't
serialize around a phantom PE bottleneck (see _build_nc). DMA queues are
split by role (Sync=x, GpSimd=h1 stores, Scalar=weights+init) so the
scan's WAR chain never queues behind bulk transfers.
"""

import numpy as np
import ml_dtypes

import concourse.bass as bass
import concourse.tile as tile
from concourse import bacc, mybir
from concourse.bass_utils import run_bass_kernel_spmd

BF16 = mybir.dt.bfloat16
F32 = mybir.dt.float32
AF = mybir.ActivationFunctionType
ALU = mybir.AluOpType

H = 50
DIN = 207
DK = 208          # D + ones row
T = 30
B = 16384
NCORES = 8
BC = B // NCORES  # 2048
NSW = 4           # sweeps per core
BS = 512          # sweep batch size
Q = 64            # quadrant offset for second lane (dir-b / sweep s2)
KH = 114          # K rows for rec matmuls / h span (0:50 real, 64:114 real)
KH1 = 115         # K rows for L1 proj (incl. ones row at 114)

# PyTorch LSTM gate order in weight rows: i, f, g, o. Our bank order: i, f, o, g.
GATE_SLICES = [slice(0, 50), slice(50, 100), slice(150, 200), slice(100, 150)]
# bank 3 (g) weight cols are scaled x2: tanh(x) = 2*sigmoid(2x) - 1
GATE_SCALE = [1.0, 1.0, 1.0, 2.0]


def _pack_weights(inp):
    f32 = np.float32
    # ---- L0 projection (+bias via ones row), [208, 4*128] ----
    w0 = np.zeros((DK, 512), f32)
    wf = np.concatenate([inp["wih0f"], inp["b0f"][:, None]], axis=1)  # [200,208]
    wb = np.concatenate([inp["wih0b"], inp["b0b"][:, None]], axis=1)
    for c, (gs, sc) in enumerate(zip(GATE_SLICES, GATE_SCALE)):
        w0[:, c * 128 + 0:c * 128 + 50] = sc * wf[gs].T
        w0[:, c * 128 + Q:c * 128 + Q + 50] = sc * wb[gs].T
    # ---- L0 recurrent, block-diag [114, 4*128] ----
    r0 = np.zeros((KH, 512), f32)
    for c, (gs, sc) in enumerate(zip(GATE_SLICES, GATE_SCALE)):
        r0[0:50, c * 128 + 0:c * 128 + 50] = sc * inp["whh0f"][gs].T
        r0[Q:Q + 50, c * 128 + Q:c * 128 + Q + 50] = sc * inp["whh0b"][gs].T
    # ---- L1 fwd projection [115, 4*64] (rows: h1f 0:50, h1b 64:114, bias 114)
    def l1_proj(wih, b):
        w = np.zeros((KH1, 256), f32)
        for c, (gs, sc) in enumerate(zip(GATE_SLICES, GATE_SCALE)):
            w[0:50, c * 64:c * 64 + 50] = sc * wih[gs, 0:50].T
            w[Q:Q + 50, c * 64:c * 64 + 50] = sc * wih[gs, 50:100].T
            w[KH, c * 64:c * 64 + 50] = sc * b[gs]
        return w
    w1 = l1_proj(inp["wih1f"], inp["b1f"])
    w1b = l1_proj(inp["wih1b"], inp["b1b"])
    # ---- L1 recurrent, block-diag per sweep pair [114, 4*128] ----
    r1 = np.zeros((KH, 512), f32)
    for c, (gs, sc) in enumerate(zip(GATE_SLICES, GATE_SCALE)):
        r1[0:50, c * 128 + 0:c * 128 + 50] = sc * inp["whh1f"][gs].T
        r1[Q:Q + 50, c * 128 + Q:c * 128 + Q + 50] = sc * inp["whh1f"][gs].T
    # ---- FC ----
    wff = np.ascontiguousarray(inp["fc_w"][:, 0:50].T)  # [50, 2]
    wfb = np.ascontiguousarray(inp["fc_w"][:, 50:100].T)
    bf = ml_dtypes.bfloat16
    return {
        "w0hi": w0[0:128].astype(bf), "w0lo": w0[128:DK].astype(bf),
        "r0": r0.astype(bf), "w1": w1.astype(bf), "w1b": w1b.astype(bf),
        "r1": r1.astype(bf), "wff": wff.astype(bf), "wfb": wfb.astype(bf),
    }


def _pack_x(x):
    # x [B, T, 207] f32 -> [T, 208, B] bf16 with ones row at d=207
    xt = np.empty((T, DK, B), dtype=ml_dtypes.bfloat16)
    xt[:, 0:DIN, :] = x.transpose(1, 2, 0).astype(ml_dtypes.bfloat16)
    xt[:, DIN, :] = ml_dtypes.bfloat16(1.0)
    return xt


def _make_hinit():
    # h1-history fixed rows, DMA'd instead of memset: row 0 -> ones row
    # (partition 114, the L1 bias row), rows 1:15 -> zeros (partitions
    # 50:64, the dead K rows between the fwd and bwd quadrants).
    hi = np.zeros((15, T, BS), dtype=ml_dtypes.bfloat16)
    hi[0] = ml_dtypes.bfloat16(1.0)
    return hi


def _build_nc():
    # The Tile scheduler's cost model times each matmul as N serial cycles,
    # but our proj matmuls run pairwise-concurrent on disjoint PE col
    # groups (quadrant packing), so the model overestimates PE time ~2x.
    # That mis-model makes the scheduler serialize the ACT stream around a
    # phantom PE bottleneck (sigma -> DVE -> tanh -> sigma chains). Halve
    # the model's PE cycle during scheduling so instruction ORDER is chosen
    # around the real ACT bottleneck; semaphores still enforce the true
    # data dependencies, so this only affects heuristic ordering.
    from concourse import hw_specs
    spec = hw_specs.TRN2Spec
    saved = (spec.PE_CYCLE, spec.PE_CYCLE_PSTATE_MID, spec.PE_CYCLE_PSTATE_LOW)
    spec.PE_CYCLE = saved[0] * 0.5
    spec.PE_CYCLE_PSTATE_MID = saved[1] * 0.5
    spec.PE_CYCLE_PSTATE_LOW = saved[2] * 0.5
    try:
        return _build_nc_inner()
    finally:
        (spec.PE_CYCLE, spec.PE_CYCLE_PSTATE_MID,
         spec.PE_CYCLE_PSTATE_LOW) = saved


def _build_nc_inner():
    nc = bacc.Bacc("TRN2", target_bir_lowering=False, debug=False)
    ap = {}
    ap["xT"] = nc.dram_tensor("xT", [T, DK, BC], BF16, kind="ExternalInput").ap()
    for name, shp in [("w0hi", [128, 512]), ("w0lo", [DK - 128, 512]),
                      ("r0", [KH, 512]), ("w1", [KH1, 256]), ("w1b", [KH1, 256]),
                      ("r1", [KH, 512]), ("wff", [50, 2]), ("wfb", [50, 2]),
                      ("hinit", [15, T, BS])]:
        ap[name] = nc.dram_tensor(name, shp, BF16, kind="ExternalInput").ap()
    out_ap = nc.dram_tensor("out", [2, BC], F32, kind="ExternalOutput").ap()

    with tile.TileContext(nc) as tc:
        with (
            tc.tile_pool(name="wts", bufs=1) as wp,
            tc.tile_pool(name="xin", bufs=3) as xp,
            tc.tile_pool(name="h1p", bufs=1) as h1p,
            tc.tile_pool(name="st", bufs=1) as sp,
            tc.tile_pool(name="gt", bufs=2) as gp,
            tc.tile_pool(name="outp", bufs=2) as op_,
        ):
            # ---- load weights to SBUF ----
            # DMA queue assignment: Sync = x loads, GpSimd = h1 stores only
            # (they gate the scan's WAR chain, so nothing may queue ahead of
            # them), Scalar = the three Phase-A weights. Everything not
            # needed until Phase B/C (L1 weights, FC weights, h1 fixed-row
            # init) is deferred into the Phase A loop on the Sync queue --
            # with all 20 issues up front the first sigma couldn't reach the
            # ACT queue head until ~31us.
            w = {}
            for name, shp in [("w0hi", [128, 512]), ("w0lo", [DK - 128, 512]),
                              ("r0", [KH, 512]), ("w1", [KH1, 256]),
                              ("w1b", [KH1, 256]), ("r1", [KH, 512]),
                              ("wff", [50, 2]), ("wfb", [50, 2])]:
                t = wp.tile(shp, BF16, tag=name, name=name)
                if name in ("w0hi", "w0lo", "r0"):
                    nc.scalar.dma_start(t[:], ap[name][:])
                else:
                    pass  # deferred below
                w[name] = t
            # FC weights staged at both quadrants (rhs base-partition match)
            for name in ("wff", "wfb"):
                w[name + "q"] = wp.tile([128, 2], BF16, tag=name + "q",
                                        name=name + "q")

            # ---- h1 history per sweep [115, T, 512]; row 114 = ones ----
            # Fixed rows come in via DMA (memsets of these ranges cost
            # ~100us of engine time and serialized the kernel start).
            h1 = [h1p.tile([128, T, BS], BF16, tag=f"h1_{s}", name=f"h1_{s}")
                  for s in range(NSW)]

            deferred = [(w["w1"][:], ap["w1"][:]), (w["w1b"][:], ap["w1b"][:]),
                        (w["r1"][:], ap["r1"][:]), (w["wff"][:], ap["wff"][:]),
                        (w["wfb"][:], ap["wfb"][:])]
            for name in ("wff", "wfb"):
                deferred.append((w[name + "q"][0:50, :], ap[name][:]))
                deferred.append((w[name + "q"][Q:Q + 50, :], ap[name][:]))
            for s in range(NSW):
                deferred.append((h1[s][114:115, :, :], ap["hinit"][0:1, :, :]))
                # rows 50:64 are never written by the scan; zero them so the
                # 0-weight K rows in layer-1 matmuls read 0 (not NaN).
                deferred.append((h1[s][50:64, :, :], ap["hinit"][1:15, :, :]))

            # ---- states (bf16 cell state: DVE 2x_1p mode) ----
            hS = [sp.tile([128, BS], BF16, tag=f"hs{s}", name=f"hs{s}") for s in range(NSW)]
            cS = [sp.tile([128, BS], BF16, tag=f"cs{s}", name=f"cs{s}") for s in range(NSW)]
            hB = [sp.tile([128, BS], BF16, tag=f"hb{p}", name=f"hb{p}") for p in range(2)]
            cB = [sp.tile([128, BS], BF16, tag=f"cb{p}", name=f"cb{p}") for p in range(2)]
            hC = [sp.tile([128, BS], BF16, tag=f"hc{p}", name=f"hc{p}") for p in range(2)]
            cC = [sp.tile([128, BS], BF16, tag=f"cc{p}", name=f"cc{p}") for p in range(2)]

            def cell_p1(P, G):
                """sigmoid over all 4 banks (g weights pre-scaled x2: bank 3
                holds sig(2*ag), converted to tanh in cell_p2). Everything
                after the sigmoid runs one pipeline slot later (cell_p2):
                the next sweep's sigma then overlaps this sweep's DVE chain,
                and when its tanh reaches the strict-FIFO ACT head the cell
                state is already updated -- the ACT engine never stalls."""
                nc.scalar.activation(G[0:KH, :, :], P[0:KH, :, :], AF.Sigmoid)

            def cell_c(G, c_t, k):
                """g = 2*sig-1 and the c update (no tanh/h yet)."""
                nc.vector.tensor_scalar(G[0:KH, 3, :], G[0:KH, 3, :],
                                        2.0, -1.0, ALU.mult, ALU.add)
                i_, f_, g_ = G[0:KH, 0, :], G[0:KH, 1, :], G[0:KH, 3, :]
                if k == 0:
                    nc.vector.tensor_mul(c_t[0:KH, :], i_, g_)
                else:
                    nc.vector.tensor_mul(g_, i_, g_)
                    nc.vector.tensor_mul(c_t[0:KH, :], f_, c_t[0:KH, :])
                    nc.vector.tensor_add(c_t[0:KH, :], c_t[0:KH, :], g_)

            def cell_p2(G, c_t, h_out, k):
                """full cell tail: c update, tanh(c) into G slot 0 (i dead
                after the ig product), h = o * th. Used by Phases B/C."""
                cell_c(G, c_t, k)
                th = G[0:KH, 0, :]
                nc.scalar.activation(th, c_t[0:KH, :], AF.Tanh)
                nc.vector.tensor_mul(h_out[0:KH, :], G[0:KH, 2, :], th)

            with tc.tile_pool(name="ps", bufs=2, space="PSUM") as pp:
                # =================== Phase A: layer 0, lockstep ===========
                pend = None  # (G, s, k) whose cell tail runs next slot

                def flush_a():
                    nonlocal pend
                    Gp, sp, kp = pend
                    cell_p2(Gp, cS[sp], hS[sp], kp)
                    # h1 history is time-aligned: bwd lane at step kp holds
                    # h_b for time T-1-kp. Stores go on the GpSimd queue so
                    # the Sync queue only feeds x loads.
                    nc.gpsimd.dma_start(h1[sp][0:50, kp, :], hS[sp][0:50, :])
                    nc.gpsimd.dma_start(h1[sp][Q:Q + 50, T - 1 - kp, :],
                                        hS[sp][Q:Q + 50, :])
                    pend = None

                for k in range(T):
                  for half in range(2):
                    # trickle out the Phase-B/C weight + h1-init transfers
                    # once the scan pipeline is running
                    if 2 <= k <= 10 and half == 1:
                        for _ in range(2):
                            if deferred:
                                dst, src = deferred.pop(0)
                                nc.sync.dma_start(dst, src)
                    # one DMA per (K-chunk, direction) covering 2 sweeps
                    hb = bass.ds(half * 2 * BS, 2 * BS)
                    xfh = xp.tile([128, 2 * BS], BF16, tag="xfh", name="xfh")
                    nc.sync.dma_start(xfh[:], ap["xT"][k, 0:128, hb])
                    xfl = xp.tile([DK - 128, 2 * BS], BF16, tag="xfl", name="xfl")
                    nc.sync.dma_start(xfl[:], ap["xT"][k, 128:DK, hb])
                    xbh = xp.tile([128, 2 * BS], BF16, tag="xbh", name="xbh")
                    nc.sync.dma_start(xbh[:], ap["xT"][T - 1 - k, 0:128, hb])
                    xbl = xp.tile([DK - 128, 2 * BS], BF16, tag="xbl", name="xbl")
                    nc.sync.dma_start(xbl[:], ap["xT"][T - 1 - k, 128:DK, hb])

                    for s in (2 * half, 2 * half + 1):
                        sl = bass.ts(s % 2, BS)
                        # Rotated interleave: adjacent MMs hit disjoint
                        # col-groups (concurrent) while same-bank writes are
                        # >=2 slots apart (concurrent drains from different
                        # col-groups into one PSUM bank corrupt it).
                        P = pp.tile([128, 4, BS], F32, tag="ps", name="ps")
                        for wt_, xfx, xbx, st in ((w["w0hi"], xfh, xbh, True),
                                                  (w["w0lo"], xfl, xbl, False)):
                            for c in range(4):
                                cb = (c + 2) % 4
                                spA = k == 0 and not st and c >= 2
                                spB = k == 0 and not st and cb < 2
                                nc.tensor.matmul(
                                    P[0:Q, c, :],
                                    wt_[:, bass.ds(c * 128, Q)], xfx[:, sl],
                                    start=st, stop=spA)
                                nc.tensor.matmul(
                                    P[Q:128, cb, :],
                                    wt_[:, bass.ds(cb * 128 + Q, Q)], xbx[:, sl],
                                    start=st, stop=spB)
                        if k > 0:
                            for c in range(4):
                                nc.tensor.matmul(
                                    P[:, c, :],
                                    w["r0"][:, bass.ts(c, 128)], hS[s][0:KH, :],
                                    start=False, stop=True)

                        G = gp.tile([128, 4, BS], BF16, tag=f"g{s}", name=f"g{s}")
                        cell_p1(P, G)
                        if pend is not None:
                            flush_a()
                        pend = (G, s, k)
                flush_a()

                # ============ Phase C: layer 1 backward (t=29 only) =======
                # Independent of Phase B (the reverse scan's first step needs
                # only h1[T-1], complete once Phase A ends) -- issued before B
                # so the scheduler can fill B's pipeline-warmup bubbles with
                # it instead of paying for it serially at the tail.
                gC = []
                for p in range(2):
                    s1, s2 = 2 * p, 2 * p + 1
                    P = pp.tile([128, 4, BS], F32, tag="ps", name="ps")
                    for c in range(4):
                        cb = (c + 2) % 4
                        nc.tensor.matmul(
                            P[0:Q, c, :], w["w1b"][:, bass.ts(c, 64)],
                            h1[s1][0:KH1, T - 1, :], start=True, stop=c >= 2)
                        nc.tensor.matmul(
                            P[Q:128, cb, :], w["w1b"][:, bass.ts(cb, 64)],
                            h1[s2][0:KH1, T - 1, :], start=True, stop=cb < 2)
                    G = gp.tile([128, 4, BS], BF16, tag=f"g{p + 2}", name=f"g{p + 2}")
                    cell_p1(P, G)
                    gC.append(G)
                for p in range(2):
                    cell_p2(gC[p], cC[p], hC[p], 0)

                # =================== Phase B: layer 1 forward =============
                pendb = None  # (G, p, k)

                def flush_b():
                    nonlocal pendb
                    Gp, pp_, kp = pendb
                    cell_p2(Gp, cB[pp_], hB[pp_], kp)
                    pendb = None

                for k in range(T):
                    for p in range(2):
                        s1, s2 = 2 * p, 2 * p + 1
                        P = pp.tile([128, 4, BS], F32, tag="ps", name="ps")
                        for c in range(4):
                            cb = (c + 2) % 4
                            nc.tensor.matmul(
                                P[0:Q, c, :], w["w1"][:, bass.ts(c, 64)],
                                h1[s1][0:KH1, k, :], start=True,
                                stop=(k == 0 and c >= 2))
                            nc.tensor.matmul(
                                P[Q:128, cb, :], w["w1"][:, bass.ts(cb, 64)],
                                h1[s2][0:KH1, k, :], start=True,
                                stop=(k == 0 and cb < 2))
                        if k > 0:
                            for c in range(4):
                                nc.tensor.matmul(
                                    P[:, c, :],
                                    w["r1"][:, bass.ts(c, 128)], hB[p][0:KH, :],
                                    start=False, stop=True)
                        G = gp.tile([128, 4, BS], BF16, tag=f"g{p}", name=f"g{p}")
                        cell_p1(P, G)
                        if pendb is not None:
                            flush_b()
                        pendb = (G, p, k)
                flush_b()

            # =================== FC ====================================
            with tc.tile_pool(name="fcp", bufs=4, space="PSUM") as fcp:
                for s in range(NSW):
                    p, qo = s // 2, (s % 2) * Q
                    F = fcp.tile([2, BS], F32, tag="fc", name="fc")
                    nc.tensor.matmul(F[:], w["wffq"][qo:qo + 50, :],
                                     hB[p][qo:qo + 50, :], start=True, stop=False)
                    nc.tensor.matmul(F[:], w["wfbq"][qo:qo + 50, :],
                                     hC[p][qo:qo + 50, :], start=False, stop=True)
                    ot = op_.tile([2, BS], F32, tag="o", name="ot")
                    nc.vector.tensor_copy(ot[:], F[:])
                    nc.sync.dma_start(out_ap[:, bass.ts(s, BS)], ot[:])

    nc.compile()
    return nc


_NC_CACHE = None


def kernel(**inputs) -> np.ndarray:
    global _NC_CACHE
    if _NC_CACHE is None:
        _NC_CACHE = _build_nc()
    nc = _NC_CACHE
    wts = _pack_weights(inputs)
    wts["hinit"] = _make_hinit()
    xt = _pack_x(np.asarray(inputs["x"], dtype=np.float32))
    in_maps = []
    for c in range(NCORES):
        m = dict(wts)
        m["xT"] = np.ascontiguousarray(xt[:, :, c * BC:(c + 1) * BC])
        in_maps.append(m)
    res = run_bass_kernel_spmd(nc, in_maps, list(range(NCORES)))
    outs = [res.results[c]["out"] for c in range(NCORES)]  # [2, BC] each
    full = np.concatenate(outs, axis=1).T  # [B, 2]
    return (full + inputs["fc_b"][None, :]).astype(np.float32)


# revision 36
# speedup vs baseline: 1.0230x; 1.0040x over previous
"""BiLSTM (2-layer, H=50, D=207, T=30, B=16384) -> FC(2) Trainium2 kernel.

Data-parallel over 8 NeuronCores (2048 batch rows each). Host pre-packs
x into feature-major bf16 layout [T, 208, B] (ones row folds biases into
the input projection) and reorders weights into PSUM-bank gate layout.

Device layout (per core, 4 sweeps of 512 batch):
  - quadrant packing: partitions 0:50 = dir-f (or sweep s1), 64:114 =
    dir-b (or sweep s2); weight columns zero-padded so junk rows are 0.
  - Phase A: layer-0 fwd+bwd scans in lockstep (fwd eats x[t=k], bwd
    x[t=29-k]); gates accumulate in PSUM [128, 4banks]; ONE sigmoid
    ACTIVATE over all 4 banks -- the g-gate weight columns are
    pre-scaled x2 on host so tanh(ag) = 2*sigmoid(2ag)-1, recovered
    with a single 4x-mode tensor_scalar on DVE. All cell-state math in
    bf16 so DVE tensor_tensor ops run in 2x_1p mode.
  - Phase B: layer-1 forward, 2 sweeps quadrant-packed per matmul pair.
  - Phase C: layer-1 backward, only t=29 (first step of reverse scan is
    all the FC needs).
  - FC on device ([2, 512] psum per sweep); bias+transpose on host.

Scheduling: the cell tail (g-fix, c update, tanh, h) is issued one
pipeline slot late so the strict-FIFO ACT engine streams
sigma(s), tanh(s-1), sigma(s+1), ... without stalling on the DVE chain;
the Tile scheduler's PE cost is halved during build so its model doesn't
serialize around a phantom PE bottleneck (see _build_nc). DMA queues are
split by role (Sync=x, GpSimd=h1 stores, Scalar=weights+init) so the
scan's WAR chain never queues behind bulk transfers.
"""

import numpy as np
import ml_dtypes

import concourse.bass as bass
import concourse.tile as tile
from concourse import bacc, mybir
from concourse.bass_utils import run_bass_kernel_spmd

BF16 = mybir.dt.bfloat16
F32 = mybir.dt.float32
AF = mybir.ActivationFunctionType
ALU = mybir.AluOpType

H = 50
DIN = 207
DK = 208          # D + ones row
T = 30
B = 16384
NCORES = 8
BC = B // NCORES  # 2048
NSW = 4           # sweeps per core
BS = 512          # sweep batch size
Q = 64            # quadrant offset for second lane (dir-b / sweep s2)
KH = 114          # K rows for rec matmuls / h span (0:50 real, 64:114 real)
KH1 = 115         # K rows for L1 proj (incl. ones row at 114)

# PyTorch LSTM gate order in weight rows: i, f, g, o. Our bank order: i, f, o, g.
GATE_SLICES = [slice(0, 50), slice(50, 100), slice(150, 200), slice(100, 150)]
# bank 3 (g) weight cols are scaled x2: tanh(x) = 2*sigmoid(2x) - 1
GATE_SCALE = [1.0, 1.0, 1.0, 2.0]


def _pack_weights(inp):
    f32 = np.float32
    # ---- L0 projection (+bias via ones row), [208, 4*128] ----
    w0 = np.zeros((DK, 512), f32)
    wf = np.concatenate([inp["wih0f"], inp["b0f"][:, None]], axis=1)  # [200,208]
    wb = np.concatenate([inp["wih0b"], inp["b0b"][:, None]], axis=1)
    for c, (gs, sc) in enumerate(zip(GATE_SLICES, GATE_SCALE)):
        w0[:, c * 128 + 0:c * 128 + 50] = sc * wf[gs].T
        w0[:, c * 128 + Q:c * 128 + Q + 50] = sc * wb[gs].T
    # ---- L0 recurrent, block-diag [114, 4*128] ----
    r0 = np.zeros((KH, 512), f32)
    for c, (gs, sc) in enumerate(zip(GATE_SLICES, GATE_SCALE)):
        r0[0:50, c * 128 + 0:c * 128 + 50] = sc * inp["whh0f"][gs].T
        r0[Q:Q + 50, c * 128 + Q:c * 128 + Q + 50] = sc * inp["whh0b"][gs].T
    # ---- L1 fwd projection [115, 4*64] (rows: h1f 0:50, h1b 64:114, bias 114)
    def l1_proj(wih, b):
        w = np.zeros((KH1, 256), f32)
        for c, (gs, sc) in enumerate(zip(GATE_SLICES, GATE_SCALE)):
            w[0:50, c * 64:c * 64 + 50] = sc * wih[gs, 0:50].T
            w[Q:Q + 50, c * 64:c * 64 + 50] = sc * wih[gs, 50:100].T
            w[KH, c * 64:c * 64 + 50] = sc * b[gs]
        return w
    w1 = l1_proj(inp["wih1f"], inp["b1f"])
    w1b = l1_proj(inp["wih1b"], inp["b1b"])
    # ---- L1 recurrent, block-diag per sweep pair [114, 4*128] ----
    r1 = np.zeros((KH, 512), f32)
    for c, (gs, sc) in enumerate(zip(GATE_SLICES, GATE_SCALE)):
        r1[0:50, c * 128 + 0:c * 128 + 50] = sc * inp["whh1f"][gs].T
        r1[Q:Q + 50, c * 128 + Q:c * 128 + Q + 50] = sc * inp["whh1f"][gs].T
    # ---- FC ----
    wff = np.ascontiguousarray(inp["fc_w"][:, 0:50].T)  # [50, 2]
    wfb = np.ascontiguousarray(inp["fc_w"][:, 50:100].T)
    bf = ml_dtypes.bfloat16
    return {
        "w0hi": w0[0:128].astype(bf), "w0lo": w0[128:DK].astype(bf),
        "r0": r0.astype(bf), "w1": w1.astype(bf), "w1b": w1b.astype(bf),
        "r1": r1.astype(bf), "wff": wff.astype(bf), "wfb": wfb.astype(bf),
    }


def _pack_x(x):
    # x [B, T, 207] f32 -> [T, 208, B] bf16 with ones row at d=207
    xt = np.empty((T, DK, B), dtype=ml_dtypes.bfloat16)
    xt[:, 0:DIN, :] = x.transpose(1, 2, 0).astype(ml_dtypes.bfloat16)
    xt[:, DIN, :] = ml_dtypes.bfloat16(1.0)
    return xt


def _make_hinit():
    # h1-history fixed rows, DMA'd instead of memset: row 0 -> ones row
    # (partition 114, the L1 bias row), rows 1:15 -> zeros (partitions
    # 50:64, the dead K rows between the fwd and bwd quadrants).
    hi = np.zeros((15, T, BS), dtype=ml_dtypes.bfloat16)
    hi[0] = ml_dtypes.bfloat16(1.0)
    return hi


def _build_nc():
    # The Tile scheduler's cost model times each matmul as N serial cycles,
    # but our proj matmuls run pairwise-concurrent on disjoint PE col
    # groups (quadrant packing), so the model overestimates PE time ~2x.
    # That mis-model makes the scheduler serialize the ACT stream around a
    # phantom PE bottleneck (sigma -> DVE -> tanh -> sigma chains). Halve
    # the model's PE cycle during scheduling so instruction ORDER is chosen
    # around the real ACT bottleneck; semaphores still enforce the true
    # data dependencies, so this only affects heuristic ordering.
    from concourse import hw_specs
    spec = hw_specs.TRN2Spec
    saved = (spec.PE_CYCLE, spec.PE_CYCLE_PSTATE_MID, spec.PE_CYCLE_PSTATE_LOW)
    spec.PE_CYCLE = saved[0] * 0.5
    spec.PE_CYCLE_PSTATE_MID = saved[1] * 0.5
    spec.PE_CYCLE_PSTATE_LOW = saved[2] * 0.5
    try:
        return _build_nc_inner()
    finally:
        (spec.PE_CYCLE, spec.PE_CYCLE_PSTATE_MID,
         spec.PE_CYCLE_PSTATE_LOW) = saved


def _build_nc_inner():
    nc = bacc.Bacc("TRN2", target_bir_lowering=False, debug=False)
    ap = {}
    ap["xT"] = nc.dram_tensor("xT", [T, DK, BC], BF16, kind="ExternalInput").ap()
    for name, shp in [("w0hi", [128, 512]), ("w0lo", [DK - 128, 512]),
                      ("r0", [KH, 512]), ("w1", [KH1, 256]), ("w1b", [KH1, 256]),
                      ("r1", [KH, 512]), ("wff", [50, 2]), ("wfb", [50, 2]),
                      ("hinit", [15, T, BS])]:
        ap[name] = nc.dram_tensor(name, shp, BF16, kind="ExternalInput").ap()
    out_ap = nc.dram_tensor("out", [2, BC], F32, kind="ExternalOutput").ap()

    with tile.TileContext(nc) as tc:
        with (
            tc.tile_pool(name="wts", bufs=1) as wp,
            tc.tile_pool(name="xin", bufs=3) as xp,
            tc.tile_pool(name="h1p", bufs=1) as h1p,
            tc.tile_pool(name="st", bufs=1) as sp,
            tc.tile_pool(name="gt", bufs=2) as gp,
            tc.tile_pool(name="outp", bufs=2) as op_,
        ):
            # ---- load weights to SBUF ----
            # DMA queue assignment: Sync = x loads, GpSimd = h1 stores only
            # (they gate the scan's WAR chain, so nothing may queue ahead of
            # them), Scalar = the three Phase-A weights. Everything not
            # needed until Phase B/C (L1 weights, FC weights, h1 fixed-row
            # init) is deferred into the Phase A loop on the Sync queue --
            # with all 20 issues up front the first sigma couldn't reach the
            # ACT queue head until ~31us.
            w = {}
            for name, shp in [("w0hi", [128, 512]), ("w0lo", [DK - 128, 512]),
                              ("r0", [KH, 512]), ("w1", [KH1, 256]),
                              ("w1b", [KH1, 256]), ("r1", [KH, 512]),
                              ("wff", [50, 2]), ("wfb", [50, 2])]:
                t = wp.tile(shp, BF16, tag=name, name=name)
                if name in ("w0hi", "w0lo", "r0"):
                    nc.scalar.dma_start(t[:], ap[name][:])
                else:
                    pass  # deferred below
                w[name] = t
            # FC weights staged at both quadrants (rhs base-partition match)
            for name in ("wff", "wfb"):
                w[name + "q"] = wp.tile([128, 2], BF16, tag=name + "q",
                                        name=name + "q")

            # ---- h1 history per sweep [115, T, 512]; row 114 = ones ----
            # Fixed rows come in via DMA (memsets of these ranges cost
            # ~100us of engine time and serialized the kernel start).
            h1 = [h1p.tile([128, T, BS], BF16, tag=f"h1_{s}", name=f"h1_{s}")
                  for s in range(NSW)]

            # Deferred transfers: (queue, dst, src). The small L1/FC weights
            # ride the Sync queue between x loads; the bulky h1-init rows
            # (430KB each) go on the Scalar queue -- on the Sync DMA queues
            # they delayed x transfers ~5us each and starved the proj
            # matmuls, vs ~0.7us of ACT issue time on Scalar.
            deferred = [("sync", w["w1"][:], ap["w1"][:]),
                        ("sync", w["w1b"][:], ap["w1b"][:]),
                        ("sync", w["r1"][:], ap["r1"][:]),
                        ("sync", w["wff"][:], ap["wff"][:]),
                        ("sync", w["wfb"][:], ap["wfb"][:])]
            for name in ("wff", "wfb"):
                deferred.append(("sync", w[name + "q"][0:50, :], ap[name][:]))
                deferred.append(("sync", w[name + "q"][Q:Q + 50, :], ap[name][:]))
            for s in range(NSW):
                deferred.append(("scalar", h1[s][114:115, :, :],
                                 ap["hinit"][0:1, :, :]))
                # rows 50:64 are never written by the scan; zero them so the
                # 0-weight K rows in layer-1 matmuls read 0 (not NaN).
                deferred.append(("scalar", h1[s][50:64, :, :],
                                 ap["hinit"][1:15, :, :]))

            # ---- states (bf16 cell state: DVE 2x_1p mode) ----
            hS = [sp.tile([128, BS], BF16, tag=f"hs{s}", name=f"hs{s}") for s in range(NSW)]
            cS = [sp.tile([128, BS], BF16, tag=f"cs{s}", name=f"cs{s}") for s in range(NSW)]
            hB = [sp.tile([128, BS], BF16, tag=f"hb{p}", name=f"hb{p}") for p in range(2)]
            cB = [sp.tile([128, BS], BF16, tag=f"cb{p}", name=f"cb{p}") for p in range(2)]
            hC = [sp.tile([128, BS], BF16, tag=f"hc{p}", name=f"hc{p}") for p in range(2)]
            cC = [sp.tile([128, BS], BF16, tag=f"cc{p}", name=f"cc{p}") for p in range(2)]

            def cell_p1(P, G):
                """sigmoid over all 4 banks (g weights pre-scaled x2: bank 3
                holds sig(2*ag), converted to tanh in cell_p2). Everything
                after the sigmoid runs one pipeline slot later (cell_p2):
                the next sweep's sigma then overlaps this sweep's DVE chain,
                and when its tanh reaches the strict-FIFO ACT head the cell
                state is already updated -- the ACT engine never stalls."""
                nc.scalar.activation(G[0:KH, :, :], P[0:KH, :, :], AF.Sigmoid)

            def cell_c(G, c_t, k):
                """g = 2*sig-1 and the c update (no tanh/h yet)."""
                nc.vector.tensor_scalar(G[0:KH, 3, :], G[0:KH, 3, :],
                                        2.0, -1.0, ALU.mult, ALU.add)
                i_, f_, g_ = G[0:KH, 0, :], G[0:KH, 1, :], G[0:KH, 3, :]
                if k == 0:
                    nc.vector.tensor_mul(c_t[0:KH, :], i_, g_)
                else:
                    nc.vector.tensor_mul(g_, i_, g_)
                    nc.vector.tensor_mul(c_t[0:KH, :], f_, c_t[0:KH, :])
                    nc.vector.tensor_add(c_t[0:KH, :], c_t[0:KH, :], g_)

            def cell_p2(G, c_t, h_out, k):
                """full cell tail: c update, tanh(c) into G slot 0 (i dead
                after the ig product), h = o * th. Used by Phases B/C."""
                cell_c(G, c_t, k)
                th = G[0:KH, 0, :]
                nc.scalar.activation(th, c_t[0:KH, :], AF.Tanh)
                nc.vector.tensor_mul(h_out[0:KH, :], G[0:KH, 2, :], th)

            with tc.tile_pool(name="ps", bufs=2, space="PSUM") as pp:
                # =================== Phase A: layer 0, lockstep ===========
                pend = None  # (G, s, k) whose cell tail runs next slot

                def flush_a():
                    nonlocal pend
                    Gp, sp, kp = pend
                    cell_p2(Gp, cS[sp], hS[sp], kp)
                    # h1 history is time-aligned: bwd lane at step kp holds
                    # h_b for time T-1-kp. Stores go on the GpSimd queue so
                    # the Sync queue only feeds x loads.
                    nc.gpsimd.dma_start(h1[sp][0:50, kp, :], hS[sp][0:50, :])
                    nc.gpsimd.dma_start(h1[sp][Q:Q + 50, T - 1 - kp, :],
                                        hS[sp][Q:Q + 50, :])
                    pend = None

                for k in range(T):
                  for half in range(2):
                    # trickle out the Phase-B/C weight + h1-init transfers
                    # once the scan pipeline is running
                    if 2 <= k <= 10 and half == 1:
                        for _ in range(2):
                            if deferred:
                                eng, dst, src = deferred.pop(0)
                                q = nc.scalar if eng == "scalar" else nc.sync
                                q.dma_start(dst, src)
                    # one DMA per (K-chunk, direction) covering 2 sweeps
                    hb = bass.ds(half * 2 * BS, 2 * BS)
                    xfh = xp.tile([128, 2 * BS], BF16, tag="xfh", name="xfh")
                    nc.sync.dma_start(xfh[:], ap["xT"][k, 0:128, hb])
                    xfl = xp.tile([DK - 128, 2 * BS], BF16, tag="xfl", name="xfl")
                    nc.sync.dma_start(xfl[:], ap["xT"][k, 128:DK, hb])
                    xbh = xp.tile([128, 2 * BS], BF16, tag="xbh", name="xbh")
                    nc.sync.dma_start(xbh[:], ap["xT"][T - 1 - k, 0:128, hb])
                    xbl = xp.tile([DK - 128, 2 * BS], BF16, tag="xbl", name="xbl")
                    nc.sync.dma_start(xbl[:], ap["xT"][T - 1 - k, 128:DK, hb])

                    for s in (2 * half, 2 * half + 1):
                        sl = bass.ts(s % 2, BS)
                        # Rotated interleave: adjacent MMs hit disjoint
                        # col-groups (concurrent) while same-bank writes are
                        # >=2 slots apart (concurrent drains from different
                        # col-groups into one PSUM bank corrupt it).
                        P = pp.tile([128, 4, BS], F32, tag="ps", name="ps")
                        for wt_, xfx, xbx, st in ((w["w0hi"], xfh, xbh, True),
                                                  (w["w0lo"], xfl, xbl, False)):
                            for c in range(4):
                                cb = (c + 2) % 4
                                spA = k == 0 and not st and c >= 2
                                spB = k == 0 and not st and cb < 2
                                nc.tensor.matmul(
                                    P[0:Q, c, :],
                                    wt_[:, bass.ds(c * 128, Q)], xfx[:, sl],
                                    start=st, stop=spA)
                                nc.tensor.matmul(
                                    P[Q:128, cb, :],
                                    wt_[:, bass.ds(cb * 128 + Q, Q)], xbx[:, sl],
                                    start=st, stop=spB)
                        if k > 0:
                            for c in range(4):
                                nc.tensor.matmul(
                                    P[:, c, :],
                                    w["r0"][:, bass.ts(c, 128)], hS[s][0:KH, :],
                                    start=False, stop=True)

                        G = gp.tile([128, 4, BS], BF16, tag=f"g{s}", name=f"g{s}")
                        cell_p1(P, G)
                        if pend is not None:
                            flush_a()
                        pend = (G, s, k)
                flush_a()

                # ============ Phase C: layer 1 backward (t=29 only) =======
                # Independent of Phase B (the reverse scan's first step needs
                # only h1[T-1], complete once Phase A ends) -- issued before B
                # so the scheduler can fill B's pipeline-warmup bubbles with
                # it instead of paying for it serially at the tail.
                gC = []
                for p in range(2):
                    s1, s2 = 2 * p, 2 * p + 1
                    P = pp.tile([128, 4, BS], F32, tag="ps", name="ps")
                    for c in range(4):
                        cb = (c + 2) % 4
                        nc.tensor.matmul(
                            P[0:Q, c, :], w["w1b"][:, bass.ts(c, 64)],
                            h1[s1][0:KH1, T - 1, :], start=True, stop=c >= 2)
                        nc.tensor.matmul(
                            P[Q:128, cb, :], w["w1b"][:, bass.ts(cb, 64)],
                            h1[s2][0:KH1, T - 1, :], start=True, stop=cb < 2)
                    G = gp.tile([128, 4, BS], BF16, tag=f"g{p + 2}", name=f"g{p + 2}")
                    cell_p1(P, G)
                    gC.append(G)
                for p in range(2):
                    cell_p2(gC[p], cC[p], hC[p], 0)

                # =================== Phase B: layer 1 forward =============
                pendb = None  # (G, p, k)

                def flush_b():
                    nonlocal pendb
                    Gp, pp_, kp = pendb
                    cell_p2(Gp, cB[pp_], hB[pp_], kp)
                    pendb = None

                for k in range(T):
                    for p in range(2):
                        s1, s2 = 2 * p, 2 * p + 1
                        P = pp.tile([128, 4, BS], F32, tag="ps", name="ps")
                        for c in range(4):
                            cb = (c + 2) % 4
                            nc.tensor.matmul(
                                P[0:Q, c, :], w["w1"][:, bass.ts(c, 64)],
                                h1[s1][0:KH1, k, :], start=True,
                                stop=(k == 0 and c >= 2))
                            nc.tensor.matmul(
                                P[Q:128, cb, :], w["w1"][:, bass.ts(cb, 64)],
                                h1[s2][0:KH1, k, :], start=True,
                                stop=(k == 0 and cb < 2))
                        if k > 0:
                            for c in range(4):
                                nc.tensor.matmul(
                                    P[:, c, :],
                                    w["r1"][:, bass.ts(c, 128)], hB[p][0:KH, :],
                                    start=False, stop=True)
                        G = gp.tile([128, 4, BS], BF16, tag=f"g{p}", name=f"g{p}")
                        cell_p1(P, G)
                        if pendb is not None:
                            flush_b()
                        pendb = (G, p, k)
                flush_b()

            # =================== FC ====================================
            with tc.tile_pool(name="fcp", bufs=4, space="PSUM") as fcp:
                for s in range(NSW):
                    p, qo = s // 2, (s % 2) * Q
                    F = fcp.tile([2, BS], F32, tag="fc", name="fc")
                    nc.tensor.matmul(F[:], w["wffq"][qo:qo + 50, :],
                                     hB[p][qo:qo + 50, :], start=True, stop=False)
                    nc.tensor.matmul(F[:], w["wfbq"][qo:qo + 50, :],
                                     hC[p][qo:qo + 50, :], start=False, stop=True)
                    ot = op_.tile([2, BS], F32, tag="o", name="ot")
                    nc.vector.tensor_copy(ot[:], F[:])
                    nc.sync.dma_start(out_ap[:, bass.ts(s, BS)], ot[:])

    nc.compile()
    return nc


_NC_CACHE = None


def kernel(**inputs) -> np.ndarray:
    global _NC_CACHE
    if _NC_CACHE is None:
        _NC_CACHE = _build_nc()
    nc = _NC_CACHE
    wts = _pack_weights(inputs)
    wts["hinit"] = _make_hinit()
    xt = _pack_x(np.asarray(inputs["x"], dtype=np.float32))
    in_maps = []
    for c in range(NCORES):
        m = dict(wts)
        m["xT"] = np.ascontiguousarray(xt[:, :, c * BC:(c + 1) * BC])
        in_maps.append(m)
    res = run_bass_kernel_spmd(nc, in_maps, list(range(NCORES)))
    outs = [res.results[c]["out"] for c in range(NCORES)]  # [2, BC] each
    full = np.concatenate(outs, axis=1).T  # [B, 2]
    return (full + inputs["fc_b"][None, :]).astype(np.float32)
